# revision 1
# baseline (speedup 1.0000x reference)
"""Cosine-similarity loss kernel for Trainium2 (8 NeuronCores, data-parallel).

Computes 1 - mean(cos_sim(cxr_row, ehr_row)) over N=65536 rows of D=512.

Strategy:
- Shard N across 8 cores (8192 rows each), host-side.
- Host casts inputs to bf16 (halves HBM traffic; final-scalar rel err
  ~2e-7 since per-element rounding noise averages out over 512*65536
  products and the row norms are scale-invariant).
- Each core streams its two 8 MiB shards through SBUF once.  Per
  128-row slice [128, 512]: dot(a,b) via fused multiply+row-reduce on
  DVE (custom AFFINE_MUL_REDUCE op), ||a||^2 / ||b||^2 via Square
  activation with row-accumulate on ACT, with the ||b||^2 work split
  between ACT and DVE to balance engine time.  All accumulations in
  fp32.
- Epilogue per core: cos = ab * sqrt(1/(aa*bb)), summed to a [128,1]
  per-partition partial.  Host sums 8x128 partials into the scalar.
"""

import numpy as np

N, D = 65536, 512
NCORES = 8
ROWS = N // NCORES          # 8192 rows per core
P = 128                     # SBUF partitions
RPP = ROWS // P             # 64 row-slices per core

_cache = {}


def _build(
    reps: int = 1,
    spt: int = 16,
    io_bufs: int = 4,
    bb_act_16: int = 5,
    aa_dve_16: int = 0,
    bcast_out: bool = False,
    dtype: str = "bf16",
):
    """Build the SPMD program.

    reps>1 repeats the whole streaming pass (for timing via slope);
    results are identical per rep.
    spt: row-slices per DMA tile; io_bufs: buffers per io tensor.
    bb_act_16: of every 16 ||b||^2 slice-ops, this many go to ACT,
    the rest to DVE (ab is always DVE, ||a||^2 always ACT).
    """
    import concourse.bacc as bacc
    import concourse.tile as tile
    from concourse import mybir

    nc = bacc.Bacc("TRN2", target_bir_lowering=False, debug=False)
    f32 = mybir.dt.float32
    dt_in = mybir.dt.bfloat16 if dtype == "bf16" else mybir.dt.float32

    a = nc.dram_tensor("a", [ROWS, D], dt_in, kind="ExternalInput")   # ehr shard
    b = nc.dram_tensor("b", [ROWS, D], dt_in, kind="ExternalInput")   # cxr shard
    out = nc.dram_tensor("out", [P, 1], f32, kind="ExternalOutput")

    # row (p*RPP + r) lives on partition p, slot r: contiguous bytes per
    # partition per tile -> large-descriptor DMAs.
    a3 = a.ap().rearrange("(p r) d -> p r d", p=P)  # [128, 64, 512]
    b3 = b.ap().rearrange("(p r) d -> p r d", p=P)

    with tile.TileContext(nc) as tc:
        with (
            tc.tile_pool(name="io", bufs=io_bufs) as io,
            tc.tile_pool(name="scratch", bufs=2) as scratch,
            tc.tile_pool(name="stats", bufs=1) as stats,
        ):
            ab_cols = stats.tile([P, RPP], f32, tag="ab")
            aa_cols = stats.tile([P, RPP], f32, tag="aa")
            # separate per-engine bb accumulators: DVE and ACT never write
            # into the same tile (avoids false cross-engine deps on
            # neighbouring 4-byte columns), merged by add in the epilogue
            bb_dve = stats.tile([P, RPP], f32, tag="bb_dve")
            bb_act = stats.tile([P, RPP], f32, tag="bb_act")
            aa_dve_cols = stats.tile([P, RPP], f32, tag="aa_dve")
            dve_dummy = stats.tile([P, 1], dt_in, tag="dve_dummy")
            nc.vector.memset(bb_dve, 0.0)
            nc.vector.memset(aa_dve_cols, 0.0)
            nc.scalar.mul(bb_act, bb_dve, 0.0)
            nc.scalar.mul(aa_cols, bb_dve, 0.0)

            nt = RPP // spt
            for rep in range(reps):
              for i in range(nt):
                at = io.tile([P, spt, D], dt_in, tag="a")
                bt = io.tile([P, spt, D], dt_in, tag="b")
                sl = slice(i * spt, (i + 1) * spt)
                nc.sync.dma_start(out=at, in_=a3[:, sl, :])
                nc.sync.dma_start(out=bt, in_=b3[:, sl, :])

                for s in range(spt):
                    col = i * spt + s
                    a_s = at[:, s, :]
                    b_s = bt[:, s, :]
                    if bcast_out:
                        scr_ab = dve_dummy.broadcast_to((P, D))
                        scr_bb = dve_dummy.broadcast_to((P, D))
                    else:
                        scr_ab = scratch.tile([P, D], dt_in, tag="scr_ab")
                        scr_bb = scratch.tile([P, D], dt_in, tag="scr_bb")
                    scr_aa = scratch.tile([P, D], dt_in, tag="scr_aa")
                    # DVE: dot(a_row, b_row) fused multiply+row-reduce
                    # (custom DVE op; the native TENSOR_TENSOR_REDUCE
                    # opcode faults this runtime's DVE sequencer)
                    nc.vector.affine_mul_reduce(
                        out=scr_ab,
                        accum_out=ab_cols[:, col : col + 1],
                        in0=a_s,
                        in1=b_s,
                        scale=1.0,
                        bias=0.0,
                    )
                    # ||b_row||^2: split between ACT and DVE for balance,
                    # spread evenly over the col sequence
                    if (col * bb_act_16) % 16 < bb_act_16:
                        nc.scalar.activation(
                            out=scr_bb,
                            in_=b_s,
                            func=mybir.ActivationFunctionType.Square,
                            accum_out=bb_act[:, col : col + 1],
                        )
                    else:
                        nc.vector.affine_mul_reduce(
                            out=scr_bb,
                            accum_out=bb_dve[:, col : col + 1],
                            in0=b_s,
                            in1=b_s,
                            scale=1.0,
                            bias=0.0,
                        )
                    # ||a_row||^2: mostly ACT, optionally a few on DVE
                    if (col * aa_dve_16) % 16 < aa_dve_16:
                        nc.vector.affine_mul_reduce(
                            out=scr_aa,
                            accum_out=aa_dve_cols[:, col : col + 1],
                            in0=a_s,
                            in1=a_s,
                            scale=1.0,
                            bias=0.0,
                        )
                    else:
                        nc.scalar.activation(
                            out=scr_aa,
                            in_=a_s,
                            func=mybir.ActivationFunctionType.Square,
                            accum_out=aa_cols[:, col : col + 1],
                        )

            # epilogue: cos = ab / sqrt(aa*bb); partial = sum over rows
            bb_cols = stats.tile([P, RPP], f32, tag="bb")
            nc.vector.tensor_add(bb_cols, bb_dve, bb_act)
            nc.vector.tensor_add(aa_cols, aa_cols, aa_dve_cols)
            denom = stats.tile([P, RPP], f32, tag="denom")
            nc.vector.tensor_mul(denom, aa_cols, bb_cols)
            nc.vector.reciprocal(denom, denom)
            nc.scalar.sqrt(denom, denom)          # 1/sqrt(aa*bb)
            cos = stats.tile([P, RPP], f32, tag="cos")
            nc.vector.tensor_mul(cos, ab_cols, denom)
            cred = stats.tile([P, 1], f32, tag="cred")
            nc.vector.tensor_reduce(
                out=cred, in_=cos, axis=mybir.AxisListType.X, op=mybir.AluOpType.add
            )
            nc.sync.dma_start(out=out.ap(), in_=cred)

    nc.compile()
    return nc


def kernel(cxr: np.ndarray, ehr: np.ndarray) -> np.ndarray:
    import ml_dtypes
    from concourse.bass_utils import run_bass_kernel_spmd

    cxr = np.asarray(cxr)
    ehr = np.asarray(ehr)
    assert cxr.shape == (N, D) and ehr.shape == (N, D)
    bf16 = ml_dtypes.bfloat16
    cxr = np.ascontiguousarray(cxr.astype(bf16))
    ehr = np.ascontiguousarray(ehr.astype(bf16))

    if "nc" not in _cache:
        _cache["nc"] = _build()
    nc = _cache["nc"]

    in_maps = [
        {
            "a": np.ascontiguousarray(ehr[i * ROWS : (i + 1) * ROWS]),
            "b": np.ascontiguousarray(cxr[i * ROWS : (i + 1) * ROWS]),
        }
        for i in range(NCORES)
    ]
    res = run_bass_kernel_spmd(nc, in_maps, core_ids=list(range(NCORES)))
    total = np.float64(0.0)
    for r in res.results:
        total += r["out"].astype(np.float64).sum()
    return np.float32(1.0 - total / N)

